# revision 55
# baseline (speedup 1.0000x reference)
"""Trainium2 Bass kernel for nn_CE_55937654063537.

Reference computation:
    b1 = conv3x3(x, g_w) + g_b            [B, 2, 512, 512]
    b2 = conv1x1(x, theta_w) + theta_b    [B, 2, 512, 512]
    m  = patch_mean(b1, 7) + patch_mean(b2, 7)   [B, 2, 7, 7]
    out = bilinear_upsample(m, 512, 512)  (half-pixel centers)

Everything is linear, so the kernel never materializes the conv outputs.
patch_mean(feat)[i, j] is (1/(H*W)) * the sum of feat over a rectangle that is
the full map minus <=3 boundary rows/cols.  Those rectangle sums are linear in
(a) the column-sum over h of x and (b) 8 boundary rows of x, so per core the
kernel is a pure streaming problem (16 MB in + 8 MB out) and the program is
organized so the two HWDGE DMA rings never sit idle:

  - the 16 MB of input streams as 16 ungated 1 MB DMAs, alternating between
    the sync and scalar rings (~10 MB each); all loads are emitted before any
    store, so each ring's FIFO drains loads first,
  - each batch's two tiny boundary-row DMAs ride the SWDGE (Pool) queue and
    never take a ring slot; 3 packed const DMAs ride it too,
  - per channel, PE column-sums the 1 MB tile against a ones vector into a
    PSUM row st, and the colsum-row stats (w-total + edge columns) are read
    straight out of that PSUM row by short DVE ops (v_late_ci) that trail the
    matmuls, keeping PSUM recycling off every critical path,
  - boundary-row stats (stage_v_early) run once the tiny DMAs land; they are
    emitted after the column sums so the in-order DVE stream serves the
    PSUM-recycling ops first,
  - tiny fp32 matmuls apply the conv-derived coefficients (boundary blocks
    [8, 14] + colsum blocks [1, 14]) giving R, then transpose + L give m^T,
  - out = A @ m @ A^T expands via float32r matmuls against the 512x7 bilinear
    matrix A; the two output channels' chains interleave across PE/DVE/ACT
    and the 1 MB stores alternate rings (co0 sync, co1 scalar), landing
    behind the loads in each ring's FIFO.

Data parallel over batch: 8 cores x 4 batches each; params replicated.
"""
import numpy as np

H = W = 512
K = 7
CIN = 4
CO = 2
BLOC = 4    # batches per core
NCORES = 8

_PROG = None          # cached Bass program (weight-independent; weights are inputs)
TRACE = False         # set True (e.g. from test.py) to profile; see LAST_EXEC_NS
LAST_EXEC_NS = None
LAST_TRACE_PATH = None


# ---------------------------------------------------------------------------
# host-side constant builders (all tiny, derived from conv weights)
# ---------------------------------------------------------------------------

def resize_mat(in_size, out_size):
    """Bilinear (half-pixel, edge-normalized) interpolation matrix [out, in],
    matching jax.image.resize(method='bilinear') for upsampling."""
    inv_scale = in_size / out_size
    sample_f = (np.arange(out_size) + 0.5) * inv_scale - 0.5
    xw = np.abs(sample_f[None, :] - np.arange(in_size)[:, None])
    weights = np.maximum(0, 1 - xw)
    total = weights.sum(axis=0, keepdims=True)
    return (weights / total).T.astype(np.float32)  # [out, in]


def build_lhsTR(g_w, g_b, theta_w, theta_b):
    """Phase-2 weight blocks (per batch; identical for every b).

    Returns (blk [4, 3, 9, 14], bias [1, 14]):
      blk[ci, dw, q, col]: coefficient of stats row q of channel ci
        (q: 0=colsum over h, 1..4=x rows 0..3, 5..8=x rows 508..511)
        in output row col = co*7 + i -> R[co, i][w] under w-shift dw.
      bias[0, col]: additive constant (applies to every w of R[col]).
    """
    gw = g_w.astype(np.float64)
    gb = g_b.astype(np.float64)
    tw = theta_w.astype(np.float64)[:, :, 0, 0]
    tb = theta_b.astype(np.float64)
    blk = np.zeros((CIN, 3, 9, 14), dtype=np.float64)
    bias = np.zeros((1, 14), dtype=np.float64)

    def add_F(col, co, dw, sign):
        for ci in range(CIN):
            blk[ci, dw, 0, col] += sign * gw[co, ci, :, dw].sum()
            blk[ci, dw, 1, col] += -sign * gw[co, ci, 2, dw]   # x row 0
            blk[ci, dw, 8, col] += -sign * gw[co, ci, 0, dw]   # x row 511
            if dw == 1:
                blk[ci, dw, 0, col] += sign * tw[co, ci]
        if dw == 1:
            bias[0, col] += sign * H * (gb[co] + tb[co])

    def add_bd(col, co, r, dw, sign):
        for ci in range(CIN):
            for dh in range(3):
                hr = r + dh - 1
                if 0 <= hr < H:
                    q = 1 + hr if hr <= 3 else 5 + (hr - (H - 4))
                    blk[ci, dw, q, col] += sign * gw[co, ci, dh, dw]
            if dw == 1:
                q = 1 + r if r <= 3 else 5 + (r - (H - 4))
                blk[ci, dw, q, col] += sign * tw[co, ci]
        if dw == 1:
            bias[0, col] += sign * (gb[co] + tb[co])

    for co in range(CO):
        for i in range(K):
            col = co * 7 + i
            for dw in range(3):
                add_F(col, co, dw, 1.0)
                if i < 3:
                    for r in range(H - 3 + i, H):
                        add_bd(col, co, r, dw, -1.0)
                elif i > 3:
                    for r in range(0, i - 3):
                        add_bd(col, co, r, dw, -1.0)
    return blk.astype(np.float32), bias.astype(np.float32)


def build_L():
    """Phase-3 lhsT [7, 7] (includes the 1/(H*W) patch-mean scale).

    Row e' order matches the R-summary columns: 0 -> total sum,
    1..3 -> R[w=0..2], 4..6 -> R[w=509..511].
    Column j yields m[i, j] = T_R - partial edge sums."""
    L = np.zeros((7, 7), dtype=np.float64)
    L[0, :] = 1.0
    for j in range(3):            # j=0,1,2: subtract tail elements w >= 509+j
        for e in range(3 + j, 6):
            L[1 + e, j] = -1.0    # e=3,4,5 -> rows 4..6
    for j in range(4, 7):         # j=4,5,6: subtract head elements w < j-3
        for e in range(0, j - 3):
            L[1 + e, j] = -1.0    # e=0,1,2 -> rows 1..3
    return (L / (H * W)).astype(np.float32)


def build_consts(g_w, g_b, theta_w, theta_b):
    """Three packed tensors so the startup needs only 3 const DMAs.

    pk14 [14, 384] f32: blk-boundary 0..167 (rows 0..7), ident 168..181,
      lmat 182..188, biasrow 189..202, biaspat 203..209, blk-colsum 210..377 (row 0).
    pk7 [7, 1024] f32r: at cols 0..511, atr[t] cols 512+128t..512+128(t+1).
    """
    blk, biasrow = build_lhsTR(g_w, g_b, theta_w, theta_b)
    A = resize_mat(K, H)          # [512, 7]
    biaspat = np.ones((1, 7), dtype=np.float32)
    biaspat[0, 0] = float(W)      # total-sum column gets bias once per w
    pk14 = np.zeros((14, 384), dtype=np.float32)
    # boundary-row coefficient blocks (old q=1..8) on partitions 0..7
    pk14[0:8, 0:168] = blk[:, :, 1:9, :].transpose(2, 0, 1, 3).reshape(8, 168)
    pk14[0:14, 168:182] = np.eye(14, dtype=np.float32)
    pk14[0:7, 182:189] = build_L()
    pk14[0:1, 189:203] = biasrow
    pk14[0:1, 203:210] = biaspat
    # colsum-row coefficients (old q=0) packed on partition 0
    pk14[0:1, 210:378] = blk[:, :, 0, :].reshape(1, 168)
    pk7 = np.zeros((7, 1024), dtype=np.float32)
    pk7[:, 0:512] = A.T
    atr = A.reshape(128, 4, K).transpose(1, 2, 0)                 # [4, 7, 128]
    for t in range(4):
        pk7[:, 512 + 128 * t:512 + 128 * (t + 1)] = atr[t]
    return {
        "pk14": pk14,
        "pk7": pk7,
        "ones128": np.ones((128, 1), dtype=np.float32),
    }


# ---------------------------------------------------------------------------
# device program
# ---------------------------------------------------------------------------

def build_program():
    import concourse.bass as bass
    import concourse.bacc as bacc
    import concourse.tile as tile
    from concourse import mybir

    f32 = mybir.dt.float32
    f32r = mybir.dt.float32r
    nc = bacc.Bacc(None, target_bir_lowering=False, enable_partition_id=False)

    xs = nc.dram_tensor("xs", [BLOC, CIN, H, W], f32r, kind="ExternalInput")
    pk14_d = nc.dram_tensor("pk14", [14, 384], f32, kind="ExternalInput")
    pk7_d = nc.dram_tensor("pk7", [7, 1024], f32r, kind="ExternalInput")
    ones_d = nc.dram_tensor("ones128", [128, 1], f32r, kind="ExternalInput")
    y = nc.dram_tensor("y", [BLOC, CO, H, W], f32, kind="ExternalOutput")

    with tile.TileContext(nc) as tc:
        with (
            tc.tile_pool(name="consts", bufs=1) as consts,
            tc.tile_pool(name="xpool", bufs=10) as xpool,
            tc.tile_pool(name="spool", bufs=4) as spool,
            tc.tile_pool(name="vpool", bufs=4) as vpool,
            tc.tile_pool(name="vcpool", bufs=4) as vcpool,
            tc.tile_pool(name="small", bufs=2) as small,
            tc.tile_pool(name="mtp", bufs=1) as mtp,
            tc.tile_pool(name="tgpool", bufs=2) as tgpool,
            tc.tile_pool(name="obuf", bufs=6) as obuf,
            tc.tile_pool(name="pstats", bufs=3, space="PSUM") as pstats,
            tc.tile_pool(name="pretm", bufs=1, space="PSUM") as pretm,
            tc.tile_pool(name="ptg", bufs=2, space="PSUM") as ptg,
            tc.tile_pool(name="poc", bufs=2, space="PSUM") as poc,
        ):
            # consts via SWDGE (Pool) so they never take an early HWDGE slot
            c_ones = consts.tile([128, 1], f32r)
            nc.gpsimd.dma_start(out=c_ones, in_=ones_d[:, :])
            c_pk14 = consts.tile([14, 384], f32)
            nc.gpsimd.dma_start(out=c_pk14, in_=pk14_d[:, :])
            c_pk7 = consts.tile([7, 1024], f32r)
            nc.gpsimd.dma_start(out=c_pk7, in_=pk7_d[:, :])

            def c_blk_bd(cidw):            # [8, 14] boundary block of blk
                return c_pk14[0:8, 14 * cidw:14 * (cidw + 1)]

            def c_blk_cs(cidw):            # [1, 14] colsum block of blk
                return c_pk14[0:1, 210 + 14 * cidw:210 + 14 * (cidw + 1)]
            c_ident = c_pk14[0:14, 168:182]
            c_lmat = c_pk14[0:7, 182:189]
            c_bias = c_pk14[0:1, 189:203]
            c_bpat = c_pk14[0:1, 203:210]
            c_at = c_pk7[0:7, 0:512]

            def c_atr(t):                  # [7, 128] block of atr
                return c_pk7[0:7, 512 + 128 * t:512 + 128 * (t + 1)]

            def zero_fill(dst, rows):      # DVE copy from pk14's zero cols
                nc.vector.tensor_copy(
                    dst, c_pk14[0:rows, 380:384].rearrange(
                        "q (c e) -> q c e", c=CIN))

            # R (cols 0:7), Et (7:63), mT (63:119) share one PSUM bank
            retm = pretm.tile([14, 119], f32, tag="retm")
            Rb = retm[0:14, 0:7]

            def Et_ps(b):
                return retm[0:7, 7 + 14 * b:7 + 14 * (b + 1)]

            def mT_ps(b):
                return retm[0:7, 63 + 14 * b:63 + 14 * (b + 1)]

            mT = mtp.tile([7, 56], f32r, tag="mT")

            # Boundary rows for every batch ride the SWDGE (Pool) queue so
            # they never take a slot on the two HWDGE load/store rings.
            Ss = []
            for b in range(BLOC):
                S = spool.tile([8, CIN, 512], f32r, tag="S")
                nc.gpsimd.dma_start(
                    out=S[0:4, :, :],
                    in_=xs[b, :, 0:4, :].rearrange("c r w -> r c w"),
                )
                nc.gpsimd.dma_start(
                    out=S[4:8, :, :],
                    in_=xs[b, :, 508:512, :].rearrange("c r w -> r c w"),
                )
                Ss.append(S)

            # 16 MB of input as 16 ungated 1 MB DMAs, alternating rings so
            # both HWDGE rings stream ~10 MB each; every load is emitted
            # before any store, so each ring's FIFO drains loads first.
            xts = []
            for b in range(BLOC):
                for ci in range(CIN):
                    xt = xpool.tile([128, 4, 512], f32r, tag="xt")
                    eng = nc.sync if (b + ci) % 2 == 0 else nc.scalar
                    eng.dma_start(
                        out=xt,
                        in_=xs[b, ci].rearrange("(p t) w -> p t w", t=4),
                    )
                    xts.append(xt)

            stss = [[None] * CIN for _ in range(BLOC)]

            def colsums(b, ci):
                # ---- phase 1: column sums for channel ci of batch b ----
                # stats stay in PSUM; the colsum-row stats (DVE) trail the
                # matmuls so they pipeline with the next channel's sums
                st = pstats.tile([1, 512], f32, tag="st")
                xt = xts[CIN * b + ci]
                for t in range(4):
                    nc.tensor.matmul(st, c_ones, xt[:, t, :],
                                     start=(t == 0), stop=(t == 3))
                stss[b][ci] = st
                v_late_ci(b, ci)

            def stage_v_early(b):
                # ---- boundary-row summaries V = [T | edges] on 32 rows ----
                # V column groups, one per w-shift dw (7 cols each):
                #  dw=0: [T-S511, 0,  S0, S1, S508, S509, S510]
                #  dw=1: [T,      S0, S1, S2, S509, S510, S511]
                #  dw=2: [T-S0,   S1, S2, S3, S510, S511, 0   ]
                S = Ss[b]
                V = vpool.tile([8, CIN, 21], f32, tag="V")
                nc.vector.reduce_sum(V[:, :, 7:8], S, axis=mybir.AxisListType.X)
                edges = bass.AP(           # S columns {0,1,2, 509,510,511}
                    tensor=S.tensor, offset=S.offset,
                    ap=[S.ap[0], S.ap[1], [509, 2], [1, 3]],
                )
                nc.vector.tensor_copy(
                    V[:, :, 8:14].rearrange("q c (g e) -> q c g e", g=2), edges)
                nc.vector.tensor_sub(V[:, :, 0:1], V[:, :, 7:8], V[:, :, 13:14])
                zero_fill(V[:, :, 1:2], 8)
                nc.vector.tensor_copy(V[:, :, 2:4], V[:, :, 8:10])
                nc.vector.tensor_copy(V[:, :, 4:7], S[:, :, 508:511])
                nc.vector.tensor_sub(V[:, :, 14:15], V[:, :, 7:8], V[:, :, 8:9])
                nc.vector.tensor_copy(V[:, :, 15:18], S[:, :, 1:4])
                nc.vector.tensor_copy(V[:, :, 18:20], V[:, :, 12:14])
                zero_fill(V[:, :, 20:21], 8)
                return V

            def v_late_ci(b, ci):
                # colsum-row stats for one channel, straight from PSUM st
                V = Vcs[b]
                st = stss[b][ci]
                nc.vector.reduce_sum(V[0:1, ci, 7:8], st,
                                     axis=mybir.AxisListType.X)
                edges = bass.AP(           # st columns {0,1,2, 509,510,511}
                    tensor=st.tensor, offset=st.offset,
                    ap=[st.ap[0], [509, 2], [1, 3]],
                )
                nc.vector.tensor_copy(
                    V[0:1, ci, 8:14].rearrange("q (g e) -> q g e", g=2), edges)
                nc.vector.tensor_copy(V[0:1, ci, 4:7], st[0:1, 508:511])
                nc.vector.tensor_copy(V[0:1, ci, 15:18], st[0:1, 1:4])
                nc.vector.tensor_sub(V[0:1, ci, 0:1], V[0:1, ci, 7:8],
                                     V[0:1, ci, 13:14])
                nc.vector.tensor_copy(V[0:1, ci, 2:4], V[0:1, ci, 8:10])
                nc.vector.tensor_sub(V[0:1, ci, 14:15], V[0:1, ci, 7:8],
                                     V[0:1, ci, 8:9])
                nc.vector.tensor_copy(V[0:1, ci, 18:20], V[0:1, ci, 12:14])

            def stage_r(b):
                # ---- phase 2b: R summaries [14, 7] via tiny fp32 matmuls ----
                # boundary-row contributions are ready long before the
                # colsum ones, so they accumulate first
                V = Vs[b]
                Vc = Vcs[b]
                nc.tensor.matmul(Rb, c_bias, c_bpat, start=True, stop=False)
                for ci in range(CIN):
                    for dw in range(3):
                        nc.tensor.matmul(
                            Rb, c_blk_bd(ci * 3 + dw),
                            V[:, ci, 7 * dw:7 * dw + 7],
                            start=False, stop=False)
                for ci in range(CIN):
                    for dw in range(3):
                        last = (ci == CIN - 1 and dw == 2)
                        nc.tensor.matmul(
                            Rb, c_blk_cs(ci * 3 + dw),
                            Vc[:, ci, 7 * dw:7 * dw + 7],
                            start=False, stop=last)

            def stage_t(b):
                # ---- phase 3a: transpose ----
                Ep = small.tile([14, 7], f32, tag="Ep")
                nc.vector.tensor_copy(Ep, Rb)
                nc.tensor.transpose(Et_ps(b), Ep, c_ident)

            def stage_m(b):
                # ---- phase 3b: m^T columns for this b ----
                Etb = small.tile([7, 14], f32, tag="Etb")
                nc.vector.tensor_copy(Etb, Et_ps(b))
                nc.tensor.matmul(mT_ps(b), c_lmat, Etb, start=True, stop=True)
                nc.vector.tensor_copy(mT[:, 14 * b:14 * b + 14], mT_ps(b))

            def stage_outs(b):
                # ---- phase 4: upsample out = A @ m @ A^T for both co ----
                # the two channels' chains interleave across PE/DVE/ACT
                tgs, obs = [], []
                for co in range(CO):
                    g = b * CO + co
                    tg_ps = ptg.tile([7, 512], f32, tag="tg_ps")
                    nc.tensor.matmul(tg_ps, mT[:, g * 7:(g + 1) * 7], c_at,
                                     start=True, stop=True)
                    tg = tgpool.tile([7, 512], f32r, tag="tg")
                    if co == 0:
                        nc.vector.tensor_copy(tg, tg_ps)
                    else:
                        nc.scalar.copy(tg, tg_ps)
                    tgs.append(tg)
                    ob = obuf.tile([128, 4, 512], f32, tag="ob")
                    obs.append(ob)
                for t in range(4):
                    for co in range(CO):
                        oc_ps = poc.tile([128, 512], f32, tag="oc")
                        nc.tensor.matmul(oc_ps, c_atr(t), tgs[co],
                                         start=True, stop=True)
                        if (2 * t + co) % 2 == 0:
                            nc.vector.tensor_copy(obs[co][:, t, :], oc_ps)
                        else:
                            nc.scalar.copy(obs[co][:, t, :], oc_ps)
                for co in range(CO):
                    oeng = nc.sync if co == 0 else nc.scalar
                    oeng.dma_start(
                        out=y[b, co].rearrange("(p t) w -> p t w", t=4),
                        in_=obs[co],
                    )

            # Boundary-row stats for every batch run during the load stream;
            # each batch's tail follows its own column sums immediately (the
            # 2 MB load cadence leaves plenty of engine slack per batch).
            Vs = [None] * BLOC
            Vcs = [None] * BLOC

            def tail(b):
                stage_r(b)
                stage_t(b)
                stage_m(b)
                stage_outs(b)

            for b in range(BLOC):
                Vc = vcpool.tile([1, CIN, 21], f32, tag="Vc")
                zero_fill(Vc[:, :, 1:2], 1)
                zero_fill(Vc[:, :, 20:21], 1)
                Vcs[b] = Vc
                for ci in range(CIN):
                    colsums(b, ci)
                # after the column sums so the in-order DVE stream serves
                # the PSUM-recycling v_late ops first; R only needs V in
                # the tail anyway
                Vs[b] = stage_v_early(b)
                tail(b)
    return nc


def _get_prog():
    global _PROG
    if _PROG is None:
        _PROG = build_program()
        _PROG.finalize()
    return _PROG


# ---------------------------------------------------------------------------
# host entry point
# ---------------------------------------------------------------------------

def kernel(x, g_w, g_b, theta_w, theta_b):
    global LAST_EXEC_NS, LAST_TRACE_PATH
    from concourse.bass_utils import run_bass_kernel_spmd

    x = np.ascontiguousarray(np.asarray(x, dtype=np.float32))
    g_w = np.asarray(g_w, dtype=np.float32)
    g_b = np.asarray(g_b, dtype=np.float32)
    theta_w = np.asarray(theta_w, dtype=np.float32)
    theta_b = np.asarray(theta_b, dtype=np.float32)

    consts = build_consts(g_w, g_b, theta_w, theta_b)
    nc = _get_prog()
    in_maps = [
        {"xs": np.ascontiguousarray(x[c * BLOC:(c + 1) * BLOC]), **consts}
        for c in range(NCORES)
    ]
    res = run_bass_kernel_spmd(nc, in_maps, core_ids=list(range(NCORES)),
                               trace=TRACE)
    LAST_EXEC_NS = res.exec_time_ns
    if TRACE and res.instructions_and_trace is not None:
        LAST_TRACE_PATH = res.instructions_and_trace[1]
    return np.concatenate([res.results[c]["y"] for c in range(NCORES)], axis=0)
